# revision 12
# baseline (speedup 1.0000x reference)
"""Two-layer GAT (PyG GATConv semantics, heads=1) on 8 Trainium2 NeuronCores.

Sharding: nodes sorted by in-degree and dealt round-robin to 8 cores, so
every core has an identical [128 dst-node, slot] grid structure (block =
128 dst nodes, L_b slots shared across cores; SPMD single program).

Layer 1 — no device gather: the host pre-expands x per edge into grid
order (xE[i] = x[src_i], bf16, transposed). hs1 per edge comes from
streaming matmuls over xE. Attention uses the factorization
p = exp(leakyrelu(es+ed)) = max(exp(es)exp(ed), exp(.2es)exp(.2ed)):
the host supplies E1=exp(es1), E2=exp(.2 es1) per edge (bf16); the
device computes r=exp(ed), r2=exp(.2 ed) per dst node once, then p via
two fused DVE ops per block (accum_out gives the softmax denominator).

Layer 2 — values depend on h, so a real gather: per-node entries
(hs2_0, hs2_1, exp(es2), exp(.2 es2)) packed 4 nodes per 256B table row
([25089, 64] f32, AllGather'd), fetched with batched gpsimd.dma_gather
(int16 super-row indices = node//4, wrapped in 16 partitions), then a
4-way mask select on DVE resolves node%4. Attention mirrors layer 1.
"""

import numpy as np
import ml_dtypes

import concourse.bacc as bacc
import concourse.bass as bass
import concourse.mybir as mybir
import concourse.tile as tile
from concourse.masks import make_identity
from concourse.bass_utils import run_bass_kernel_spmd

BF16 = mybir.dt.bfloat16
F32 = mybir.dt.float32
I16 = mybir.dt.int16

P = 128
NCORES = 8
F_IN = 128
HID = 64
OUT = 2
TW2 = 4          # per-node payload: hs2_0 hs2_1 q1 q2 (f32)
EW = 16          # f32 slots per node entry in the gather table (4 used)
NPR = 4          # nodes per 256B table row
ROWE = NPR * EW  # 64 f32 per table row
PACK = 64        # layer-1 grid columns per work pack
SUBB = 7         # layer-1 psum batch (columns per PSUM tile)
GC = 8           # grid columns per dma_gather (1024 idx ring limit)
CC = 64          # layer-2 select chunk (grid columns per staging tile)
ES_NEG = -200.0


def preprocess(x, edge_index, v1s, cfg):
    """Host preprocessing: sharding, grid layout, expanded features."""
    N, CN, NB = cfg["N"], cfg["CN"], cfg["NB"]
    NTOT = NCORES * CN
    src = np.asarray(edge_index[0], dtype=np.int64)
    dst = np.asarray(edge_index[1], dtype=np.int64)
    E = src.shape[0]

    deg = np.bincount(dst, minlength=N)
    order = np.argsort(-deg, kind="stable")
    old_of_new = np.full(NTOT, -1, dtype=np.int64)
    s = np.arange(N)
    old_of_new[(s % NCORES) * CN + s // NCORES] = order
    new_of_old = np.empty(N, dtype=np.int64)
    new_of_old[order] = (s % NCORES) * CN + s // NCORES

    deg_new = np.zeros(NTOT, dtype=np.int64)
    valid = old_of_new >= 0
    deg_new[valid] = deg[old_of_new[valid]]
    Lb = np.maximum(deg_new.reshape(NCORES, NB, P).max(axis=(0, 2)), 1)
    offs = np.concatenate([[0], np.cumsum(Lb)])
    S = int(offs[-1])
    DUMMY = NTOT

    src_new = new_of_old[src]
    dst_new = new_of_old[dst]
    eo = np.argsort(dst_new, kind="stable")
    sd, ss = dst_new[eo], src_new[eo]
    starts = np.concatenate([[0], np.flatnonzero(np.diff(sd)) + 1])
    counts = np.diff(np.concatenate([starts, [E]]))
    rank = np.arange(E) - np.repeat(starts, counts)
    cc, qq = sd // CN, sd % CN
    bb, pp = qq // P, qq % P
    col = offs[bb] + rank

    esrc = np.full((NCORES, P, S), -1, dtype=np.int64)   # -1 = pad slot
    esrc[cc, pp, col] = ss

    meta = dict(Lb=[int(v) for v in Lb], offs=[int(v) for v in offs],
                S=S, CN=CN, NB=NB, NTOT=NTOT)
    packs = []
    cur, cur_cols, col0 = [], 0, 0
    for b, L in enumerate(meta["Lb"]):
        if cur_cols + L > PACK:
            packs.append((col0, cur))
            col0 += cur_cols
            cur, cur_cols = [], 0
        cur.append(b)
        cur_cols += L
    packs.append((col0, cur))
    meta["packs"] = packs

    bf = ml_dtypes.bfloat16
    xf = np.asarray(x, dtype=np.float32)
    u = (v1s * (ES_NEG / float(v1s @ v1s))).astype(np.float32)
    xpad = np.zeros((NTOT, F_IN), dtype=np.float32)
    xpad[valid] = xf[old_of_new[valid]]
    xET, E1E, E2E, xsT, gidx16, mskE = [], [], [], [], [], []
    for c in range(NCORES):
        e2 = esrc[c].T.reshape(-1)                   # [S*128] column-major
        xe = np.where(e2[:, None] >= 0, xpad[np.maximum(e2, 0)], u[None, :])
        xET.append(np.ascontiguousarray(xe.T.astype(bf)))      # [128F, S*P]
        es1 = (xe.astype(np.float64) @ v1s.astype(np.float64))
        es1 = es1.reshape(S, P).T                               # [128p, S]
        E1E.append(np.exp(es1).astype(bf))
        E2E.append(np.exp(0.2 * es1).astype(bf))
        xs = xpad[c * CN:(c + 1) * CN]
        xsT.append(np.ascontiguousarray(xs.T.astype(bf)))       # [128F, CN]
        gid = np.where(esrc[c] >= 0, esrc[c], 0)                # [P, S]
        sup = (gid // NPR).astype(np.int16)
        w16 = sup.T.reshape(-1).reshape(-1, 16)                 # [S*8, 16]
        idx = np.empty((128, S * 8), np.int16)
        for g in range(8):
            idx[g * 16:(g + 1) * 16, :] = w16.T
        gidx16.append(idx)
        wmod = gid % NPR
        msk = np.zeros((P, NPR * S), dtype=bf)
        for k in range(NPR):
            # pad slots (esrc<0) get all-zero masks -> exact 0 after select
            msk[:, k * S:(k + 1) * S] = (wmod == k) & (esrc[c] >= 0)
        mskE.append(msk)
    return dict(xET=xET, E1E=E1E, E2E=E2E, xsT=xsT, gidx16=gidx16,
                mskE=mskE, old_of_new=old_of_new), meta


def build_program(meta):
    NB, CN, S = meta["NB"], meta["CN"], meta["S"]
    NTOT = meta["NTOT"]
    NROWS = NTOT // NPR                 # 25088 table rows + 1 dummy
    SH = CN // NPR                      # shard rows per core
    Lb, offs, packs = meta["Lb"], meta["offs"], meta["packs"]
    EXP = mybir.ActivationFunctionType.Exp

    nc = bacc.Bacc("TRN2", target_bir_lowering=False, debug=False,
                   num_devices=NCORES)

    xET_d = nc.declare_dram_parameter("xET", [P, S * P], BF16, isOutput=False)
    E1_d = nc.declare_dram_parameter("E1E", [P, S], BF16, isOutput=False)
    E2_d = nc.declare_dram_parameter("E2E", [P, S], BF16, isOutput=False)
    xsT_d = nc.declare_dram_parameter("xsT", [P, CN], BF16, isOutput=False)
    gidx_d = nc.declare_dram_parameter("gidx16", [P, S * 8], I16,
                                       isOutput=False)
    msk_d = nc.declare_dram_parameter("mskE", [P, NPR * S], BF16,
                                      isOutput=False)
    w1_d = nc.declare_dram_parameter("w1", [P, HID], BF16, isOutput=False)
    wl1_d = nc.declare_dram_parameter("wl1", [P, HID + 1], BF16, isOutput=False)
    w2_d = nc.declare_dram_parameter("w2", [HID, OUT + 4], BF16, isOutput=False)
    bc1_d = nc.declare_dram_parameter("bc1", [1, HID], F32, isOutput=False)
    bc2_d = nc.declare_dram_parameter("bc2", [1, OUT], F32, isOutput=False)
    out_d = nc.declare_dram_parameter("out", [CN, OUT], F32, isOutput=True)

    tbl4s = nc.dram_tensor("tbl4s", [SH, ROWE], F32)
    tbl4g = nc.dram_tensor("tbl4g", [NROWS, ROWE], F32)

    def ap(t, off, dims):
        return bass.AP(t[:].tensor, off, dims)

    with tile.TileContext(nc) as tc:
        with (
            tc.tile_pool(name="res", bufs=1) as res,
            tc.tile_pool(name="wrk", bufs=3) as wrk,
            tc.tile_pool(name="big", bufs=2) as big,
            tc.tile_pool(name="ps", bufs=3, space="PSUM") as psp,
            tc.tile_pool(name="ps2", bufs=2, space="PSUM") as psp2,
        ):
            w1_sb = res.tile([P, HID], BF16)
            nc.sync.dma_start(w1_sb[:], w1_d[:])
            wl1_sb = res.tile([P, HID + 1], BF16)
            nc.sync.dma_start(wl1_sb[:], wl1_d[:])
            w2_sb = res.tile([HID, OUT + 4], BF16)
            nc.sync.dma_start(w2_sb[:], w2_d[:])
            bc1_sb = res.tile([P, HID], F32)
            nc.sync.dma_start(bc1_sb[:], ap(bc1_d, 0, [[0, P], [1, HID]]))
            bc2_sb = res.tile([P, OUT], F32)
            nc.sync.dma_start(bc2_sb[:], ap(bc2_d, 0, [[0, P], [1, OUT]]))
            ident = res.tile([P, P], F32)
            make_identity(nc, ident[:])
            E1sb = res.tile([P, S], BF16)
            nc.sync.dma_start(E1sb[:], E1_d[:])
            E2sb = res.tile([P, S], BF16)
            nc.sync.dma_start(E2sb[:], E2_d[:])
            gidx_sb = res.tile([P, S * 8], I16)
            nc.sync.dma_start(gidx_sb[:], gidx_d[:])
            msk_sb = res.tile([P, NPR * S], BF16)
            nc.sync.dma_start(msk_sb[:], msk_d[:])

            linbuf = res.tile([P, NB, HID], F32)
            lin2buf = res.tile([P, NB, OUT], F32)
            edl = res.tile([P, NB], F32)
            R1 = res.tile([P, NB], F32)
            R2 = res.tile([P, NB], F32)
            ed2l = res.tile([P, NB], F32)
            R21 = res.tile([P, NB], F32)
            R22 = res.tile([P, NB], F32)
            s1 = res.tile([P, NB], F32)
            s2 = res.tile([P, NB], F32)
            hT = res.tile([HID, CN], BF16)
            outsb = res.tile([P, NB, OUT], F32)
            G2 = res.tile([P, S, TW2], BF16)

            # ---- phase A: shard lin1 / ed1 -------------------------------
            for b in range(NB):
                xs_sb = wrk.tile([P, P], BF16, tag="xs")
                nc.sync.dma_start(xs_sb[:], xsT_d[:, b * P:(b + 1) * P])
                psB = psp.tile([P, SUBB * HID], F32, tag="ps")
                nc.tensor.matmul(psB[:, 0:HID + 1], xs_sb[:], wl1_sb[:],
                                 start=True, stop=True)
                nc.vector.tensor_tensor(out=linbuf[:, b, :],
                                        in0=psB[:, 0:HID], in1=bc1_sb[:],
                                        op=mybir.AluOpType.add)
                nc.scalar.copy(edl[:, b:b + 1], psB[:, HID:HID + 1])
            nc.scalar.activation(R1[:], edl[:], EXP)
            nc.scalar.activation(R2[:], edl[:], EXP, scale=0.2)

            # ---- phase B: layer 1 ----------------------------------------
            for col0, blocks in packs:
                cols = sum(Lb[b] for b in blocks)
                G = big.tile([P, PACK, HID], BF16, tag="G")
                for c0 in range(0, cols, SUBB):
                    nsub = min(SUBB, cols - c0)
                    xe_sb = wrk.tile([P, SUBB * P], BF16, tag="xe")
                    nc.sync.dma_start(
                        xe_sb[:, 0:nsub * P],
                        xET_d[:, (col0 + c0) * P:(col0 + c0 + nsub) * P])
                    psA = psp.tile([P, SUBB * HID], F32, tag="ps")
                    for j in range(nsub):
                        nc.tensor.matmul(psA[:, j * HID:(j + 1) * HID],
                                         xe_sb[:, j * P:(j + 1) * P],
                                         w1_sb[:], start=True, stop=True)
                    nc.scalar.copy(
                        bass.AP(G[:].tensor, G[:].offset + c0 * HID,
                                [G[:].ap[0], [1, nsub * HID]]),
                        psA[:, 0:nsub * HID])
                Pp = wrk.tile([P, PACK], BF16, tag="Pp")
                for b in blocks:
                    o, L = offs[b], Lb[b]
                    oo = o - col0
                    t1 = wrk.tile([P, PACK], F32, tag="t1")
                    nc.vector.scalar_tensor_tensor(
                        out=t1[:, 0:L], in0=E2sb[:, o:o + L],
                        scalar=R2[:, b:b + 1], in1=E2sb[:, o:o + L],
                        op0=mybir.AluOpType.mult,
                        op1=mybir.AluOpType.bypass)
                    nc.vector.scalar_tensor_tensor(
                        out=Pp[:, oo:oo + L], in0=E1sb[:, o:o + L],
                        scalar=R1[:, b:b + 1], in1=t1[:, 0:L],
                        op0=mybir.AluOpType.mult, op1=mybir.AluOpType.max,
                        accum_out=s1[:, b:b + 1])
                W = big.tile([P, PACK, HID], BF16, tag="W")
                nc.vector.tensor_tensor(
                    out=W[:, 0:cols, :], in0=G[:, 0:cols, :],
                    in1=bass.AP(Pp[:].tensor, Pp[:].offset,
                                [Pp[:].ap[0], [1, cols], [0, HID]]),
                    op=mybir.AluOpType.mult)
                for b in blocks:
                    o, L = offs[b], Lb[b]
                    oo = o - col0
                    # contiguous tree reduction over the L slot columns
                    n = L
                    while n > 1:
                        h = n // 2
                        nc.vector.tensor_tensor(
                            out=bass.AP(W[:].tensor, W[:].offset + oo * HID,
                                        [W[:].ap[0], [1, h * HID]]),
                            in0=bass.AP(W[:].tensor, W[:].offset + oo * HID,
                                        [W[:].ap[0], [1, h * HID]]),
                            in1=bass.AP(W[:].tensor,
                                        W[:].offset + (oo + n - h) * HID,
                                        [W[:].ap[0], [1, h * HID]]),
                            op=mybir.AluOpType.add)
                        n -= h
                    acc = wrk.tile([P, HID], F32, tag="acc")
                    nc.vector.tensor_copy(
                        acc[:], bass.AP(W[:].tensor, W[:].offset + oo * HID,
                                        [W[:].ap[0], [1, HID]]))
                    rec = wrk.tile([P, 1], F32, tag="rec")
                    nc.vector.reciprocal(rec[:], s1[:, b:b + 1])
                    th = wrk.tile([P, HID], F32, tag="th")
                    nc.vector.scalar_tensor_tensor(
                        out=th[:], in0=acc[:], scalar=rec[:, 0:1],
                        in1=linbuf[:, b, :], op0=mybir.AluOpType.mult,
                        op1=mybir.AluOpType.add)
                    psT = psp2.tile([HID, P], F32, tag="pst")
                    nc.tensor.transpose(out=psT[:], in_=th[:],
                                        identity=ident[:])
                    nc.scalar.activation(hT[:, b * P:(b + 1) * P], psT[:],
                                         mybir.ActivationFunctionType.Relu)
                    # phase C interleaved: layer-2 table entries for block b
                    psC = psp.tile([P, SUBB * HID], F32, tag="ps")
                    nc.tensor.matmul(psC[:, 0:OUT + 4],
                                     hT[:, b * P:(b + 1) * P], w2_sb[:],
                                     start=True, stop=True)
                    e4 = wrk.tile([P, EW], F32, tag="e4")
                    nc.vector.tensor_copy(e4[:, 0:2], psC[:, 0:2])
                    nc.scalar.activation(e4[:, 2:3], psC[:, 2:3], EXP)
                    nc.scalar.activation(e4[:, 3:4], psC[:, 2:3], EXP,
                                         scale=0.2)
                    nc.vector.memset(e4[:, TW2:EW], 0.0)
                    nc.sync.dma_start(
                        ap(tbl4s, b * P * EW, [[EW, P], [1, EW]]), e4[:])
                    nc.scalar.copy(ed2l[:, b:b + 1], psC[:, OUT + 1:OUT + 2])
                    nc.vector.tensor_tensor(out=lin2buf[:, b, :],
                                            in0=psC[:, OUT + 2:OUT + 4],
                                            in1=bc2_sb[:],
                                            op=mybir.AluOpType.add)
            nc.scalar.activation(R21[:], ed2l[:], EXP)
            nc.scalar.activation(R22[:], ed2l[:], EXP, scale=0.2)
            nc.gpsimd.collective_compute(
                "AllGather", mybir.AluOpType.bypass,
                replica_groups=[list(range(NCORES))],
                ins=[tbl4s[:]], outs=[tbl4g[:]])

            # ---- phase D: layer 2, attention interleaved into the gather -
            P2f = res.tile([P, S], BF16)
            done_b, done_pk = 0, 0
            for col0 in range(0, S, CC):
                kc = min(CC, S - col0)
                gbuf = big.tile([P, CC, ROWE], F32, tag="gb")
                for g0 in range(0, kc, GC):
                    gk = min(GC, kc - g0)
                    nc.gpsimd.dma_gather(
                        out_ap=gbuf[:, g0:g0 + gk, :], in_ap=tbl4g[:],
                        idxs_ap=gidx_sb[:, (col0 + g0) * 8:
                                        (col0 + g0 + gk) * 8],
                        num_idxs=gk * P, num_idxs_reg=gk * P,
                        elem_size=ROWE)
                g2o = bass.AP(G2[:].tensor, G2[:].offset + col0 * TW2,
                              [G2[:].ap[0], [TW2, kc], [1, TW2]])
                for k in range(NPR):
                    src = bass.AP(gbuf[:].tensor, gbuf[:].offset + k * EW,
                                  [gbuf[:].ap[0], [ROWE, kc], [1, TW2]])
                    mk = bass.AP(msk_sb[:].tensor,
                                 msk_sb[:].offset + k * S + col0,
                                 [msk_sb[:].ap[0], [1, kc], [0, TW2]])
                    if k == 0:
                        nc.vector.tensor_tensor(out=g2o, in0=src, in1=mk,
                                                op=mybir.AluOpType.mult)
                    else:
                        tt = wrk.tile([P, CC * TW2], F32, tag="tt")
                        tv = bass.AP(tt[:].tensor, tt[:].offset,
                                     [tt[:].ap[0], [TW2, kc], [1, TW2]])
                        nc.vector.tensor_tensor(out=tv, in0=src, in1=mk,
                                                op=mybir.AluOpType.mult)
                        nc.vector.tensor_tensor(out=g2o, in0=g2o, in1=tv,
                                                op=mybir.AluOpType.add)
                chunk_end = col0 + kc
                while done_b < NB and offs[done_b] + Lb[done_b] <= chunk_end:
                    b = done_b
                    o, L = offs[b], Lb[b]
                    q1v = bass.AP(G2[:].tensor, G2[:].offset + o * TW2 + 2,
                                  [G2[:].ap[0], [TW2, L]])
                    q2v = bass.AP(G2[:].tensor, G2[:].offset + o * TW2 + 3,
                                  [G2[:].ap[0], [TW2, L]])
                    t1 = wrk.tile([P, PACK], F32, tag="t1")
                    nc.vector.scalar_tensor_tensor(
                        out=t1[:, 0:L], in0=q2v, scalar=R22[:, b:b + 1],
                        in1=q2v, op0=mybir.AluOpType.mult,
                        op1=mybir.AluOpType.bypass)
                    nc.vector.scalar_tensor_tensor(
                        out=P2f[:, o:o + L], in0=q1v,
                        scalar=R21[:, b:b + 1], in1=t1[:, 0:L],
                        op0=mybir.AluOpType.mult, op1=mybir.AluOpType.max,
                        accum_out=s2[:, b:b + 1])
                    nc.vector.tensor_scalar_max(s2[:, b:b + 1],
                                                s2[:, b:b + 1], 1e-30)
                    done_b += 1
                while done_pk < len(packs):
                    col0p, blocksp = packs[done_pk]
                    colsp = sum(Lb[b] for b in blocksp)
                    if col0p + colsp > chunk_end or \
                            blocksp[-1] >= done_b:
                        break
                    W2t = wrk.tile([P, PACK, OUT], F32, tag="W2t")
                    nc.vector.tensor_tensor(
                        out=W2t[:, 0:colsp, :],
                        in0=bass.AP(G2[:].tensor,
                                    G2[:].offset + col0p * TW2,
                                    [G2[:].ap[0], [TW2, colsp], [1, OUT]]),
                        in1=bass.AP(P2f[:].tensor, P2f[:].offset + col0p,
                                    [P2f[:].ap[0], [1, colsp], [0, OUT]]),
                        op=mybir.AluOpType.mult)
                    for b in blocksp:
                        o, L = offs[b], Lb[b]
                        oo = o - col0p
                        acc2 = wrk.tile([P, OUT], F32, tag="acc2")
                        wv = bass.AP(W2t[:].tensor, W2t[:].offset + oo * OUT,
                                     [W2t[:].ap[0], [1, OUT], [OUT, L]])
                        nc.vector.tensor_reduce(out=acc2[:], in_=wv,
                                                axis=mybir.AxisListType.X,
                                                op=mybir.AluOpType.add)
                        rec = wrk.tile([P, 1], F32, tag="rec")
                        nc.vector.reciprocal(rec[:], s2[:, b:b + 1])
                        to = wrk.tile([P, OUT], F32, tag="to")
                        nc.vector.scalar_tensor_tensor(
                            out=to[:], in0=acc2[:], scalar=rec[:, 0:1],
                            in1=lin2buf[:, b, :], op0=mybir.AluOpType.mult,
                            op1=mybir.AluOpType.add)
                        nc.scalar.activation(
                            outsb[:, b, :], to[:],
                            mybir.ActivationFunctionType.Sigmoid)
                    done_pk += 1

            nc.sync.dma_start(
                ap(out_d, 0, [[OUT, P], [OUT * P, NB], [1, OUT]]), outsb[:])

    nc.compile()
    return nc


def _host_params(W1_src, att1_src, W1_dst, att1_dst, b1, Wl1, bl1,
                 W2_src, att2_src, W2_dst, att2_dst, b2, Wl2, bl2):
    bf = ml_dtypes.bfloat16
    v1s = (np.asarray(W1_src, np.float64)
           @ np.asarray(att1_src, np.float64)[0]).astype(np.float32)
    v1d = (W1_dst @ att1_dst[0]).astype(np.float32)
    v2s = (W2_src @ att2_src[0]).astype(np.float32)
    v2d = (W2_dst @ att2_dst[0]).astype(np.float32)
    # w2 cols: hs2_0 hs2_1 es2 ed2 lin2_0 lin2_1  -> OUT+4 wide
    w2 = np.concatenate([W2_src, v2s[:, None], v2d[:, None], Wl2], axis=1)
    wl1 = np.concatenate([Wl1, v1d[:, None]], axis=1)
    return dict(
        w1=np.asarray(W1_src).astype(bf), wl1=wl1.astype(bf),
        w2=w2.astype(bf),
        bc1=(b1 + bl1).reshape(1, HID).astype(np.float32),
        bc2=(b2 + bl2).reshape(1, OUT).astype(np.float32)), v1s


_CACHE = {}


def run(x, edge_index, params, cfg, runner=None):
    pp, v1s = _host_params(**params)
    host, meta = preprocess(x, edge_index, v1s, cfg)
    key = (tuple(meta["Lb"]), meta["CN"])
    if key not in _CACHE:
        _CACHE[key] = build_program(meta)
    nc = _CACHE[key]
    in_maps = []
    for c in range(NCORES):
        m = dict(pp)
        m["xET"] = host["xET"][c]
        m["E1E"] = host["E1E"][c]
        m["E2E"] = host["E2E"][c]
        m["xsT"] = host["xsT"][c]
        m["gidx16"] = host["gidx16"][c]
        m["mskE"] = host["mskE"][c]
        in_maps.append(m)
    if runner is None:
        res = run_bass_kernel_spmd(nc, in_maps, list(range(NCORES)))
        outs = [r["out"] for r in res.results]
    else:
        outs, res = runner(nc, in_maps)
    full = np.concatenate(outs, axis=0)
    y = np.zeros((cfg["N"], OUT), dtype=np.float32)
    valid = host["old_of_new"] >= 0
    y[host["old_of_new"][valid]] = full[valid]
    return y, res


def kernel(x, edge_index, W1_src, W1_dst, att1_src, att1_dst, b1, Wl1, bl1,
           W2_src, W2_dst, att2_src, att2_dst, b2, Wl2, bl2):
    cfg = dict(N=100000, CN=12544, NB=98)
    params = dict(W1_src=np.asarray(W1_src), att1_src=np.asarray(att1_src),
                  W1_dst=np.asarray(W1_dst), att1_dst=np.asarray(att1_dst),
                  b1=np.asarray(b1), Wl1=np.asarray(Wl1), bl1=np.asarray(bl1),
                  W2_src=np.asarray(W2_src), att2_src=np.asarray(att2_src),
                  W2_dst=np.asarray(W2_dst), att2_dst=np.asarray(att2_dst),
                  b2=np.asarray(b2), Wl2=np.asarray(Wl2), bl2=np.asarray(bl2))
    y, _ = run(np.asarray(x), np.asarray(edge_index), params, cfg)
    return y


# revision 13
# speedup vs baseline: 1.0689x; 1.0689x over previous
"""Two-layer GAT (PyG GATConv semantics, heads=1) on 8 Trainium2 NeuronCores.

Sharding: nodes sorted by in-degree and dealt round-robin to 8 cores, so
every core has an identical [128 dst-node, slot] grid structure (block =
128 dst nodes, L_b slots shared across cores; SPMD single program).

Layer 1 — no device gather: the host pre-expands x per edge into grid
order (xE[i] = x[src_i], bf16, transposed). hs1 per edge comes from
streaming matmuls over xE. Attention uses the factorization
p = exp(leakyrelu(es+ed)) = max(exp(es)exp(ed), exp(.2es)exp(.2ed)):
the host supplies E1=exp(es1), E2=exp(.2 es1) per edge (bf16); the
device computes r=exp(ed), r2=exp(.2 ed) per dst node once, then p via
two fused DVE ops per block (accum_out gives the softmax denominator).

Layer 2 — values depend on h, so a real gather: per-node entries
(hs2_0, hs2_1, exp(es2), exp(.2 es2)) packed 4 nodes per 256B table row
([25089, 64] f32, AllGather'd), fetched with batched gpsimd.dma_gather
(int16 super-row indices = node//4, wrapped in 16 partitions), then a
4-way mask select on DVE resolves node%4. Attention mirrors layer 1.
"""

import numpy as np
import ml_dtypes

import concourse.bacc as bacc
import concourse.bass as bass
import concourse.mybir as mybir
import concourse.tile as tile
from concourse.masks import make_identity
from concourse.bass_utils import run_bass_kernel_spmd

BF16 = mybir.dt.bfloat16
F32 = mybir.dt.float32
I16 = mybir.dt.int16

P = 128
NCORES = 8
F_IN = 128
HID = 64
OUT = 2
TW2 = 4          # per-node payload: hs2_0 hs2_1 q1 q2 (f32)
EW = 16          # f32 slots per node entry in the gather table (4 used)
NPR = 4          # nodes per 256B table row
ROWE = NPR * EW  # 64 f32 per table row
PACK = 64        # layer-1 grid columns per work pack
SUBB = 7         # layer-1 psum batch (columns per PSUM tile)
GC = 8           # grid columns per dma_gather (1024 idx ring limit)
CC = 64          # layer-2 select chunk (grid columns per staging tile)
ES_NEG = -200.0


def preprocess(x, edge_index, v1s, cfg):
    """Host preprocessing: sharding, grid layout, expanded features."""
    N, CN, NB = cfg["N"], cfg["CN"], cfg["NB"]
    NTOT = NCORES * CN
    src = np.asarray(edge_index[0], dtype=np.int64)
    dst = np.asarray(edge_index[1], dtype=np.int64)
    E = src.shape[0]

    deg = np.bincount(dst, minlength=N)
    order = np.argsort(-deg, kind="stable")
    old_of_new = np.full(NTOT, -1, dtype=np.int64)
    s = np.arange(N)
    old_of_new[(s % NCORES) * CN + s // NCORES] = order
    new_of_old = np.empty(N, dtype=np.int64)
    new_of_old[order] = (s % NCORES) * CN + s // NCORES

    deg_new = np.zeros(NTOT, dtype=np.int64)
    valid = old_of_new >= 0
    deg_new[valid] = deg[old_of_new[valid]]
    Lb = np.maximum(deg_new.reshape(NCORES, NB, P).max(axis=(0, 2)), 1)
    offs = np.concatenate([[0], np.cumsum(Lb)])
    S = int(offs[-1])
    DUMMY = NTOT

    src_new = new_of_old[src]
    dst_new = new_of_old[dst]
    eo = np.argsort(dst_new, kind="stable")
    sd, ss = dst_new[eo], src_new[eo]
    starts = np.concatenate([[0], np.flatnonzero(np.diff(sd)) + 1])
    counts = np.diff(np.concatenate([starts, [E]]))
    rank = np.arange(E) - np.repeat(starts, counts)
    cc, qq = sd // CN, sd % CN
    bb, pp = qq // P, qq % P
    col = offs[bb] + rank

    esrc = np.full((NCORES, P, S), -1, dtype=np.int64)   # -1 = pad slot
    esrc[cc, pp, col] = ss

    meta = dict(Lb=[int(v) for v in Lb], offs=[int(v) for v in offs],
                S=S, CN=CN, NB=NB, NTOT=NTOT)
    packs = []
    cur, cur_cols, col0 = [], 0, 0
    for b, L in enumerate(meta["Lb"]):
        if cur_cols + L > PACK:
            packs.append((col0, cur))
            col0 += cur_cols
            cur, cur_cols = [], 0
        cur.append(b)
        cur_cols += L
    packs.append((col0, cur))
    meta["packs"] = packs

    bf = ml_dtypes.bfloat16
    xf = np.asarray(x, dtype=np.float32)
    u = (v1s * (ES_NEG / float(v1s @ v1s))).astype(np.float32)
    xpad = np.zeros((NTOT, F_IN), dtype=np.float32)
    xpad[valid] = xf[old_of_new[valid]]
    xET, E1E, E2E, xsT, gidx16, mskE = [], [], [], [], [], []
    for c in range(NCORES):
        e2 = esrc[c].T.reshape(-1)                   # [S*128] column-major
        xe = np.where(e2[:, None] >= 0, xpad[np.maximum(e2, 0)], u[None, :])
        xET.append(np.ascontiguousarray(xe.T.astype(bf)))      # [128F, S*P]
        es1 = (xe.astype(np.float64) @ v1s.astype(np.float64))
        es1 = es1.reshape(S, P).T                               # [128p, S]
        E1E.append(np.exp(es1).astype(bf))
        E2E.append(np.exp(0.2 * es1).astype(bf))
        xs = xpad[c * CN:(c + 1) * CN]
        xsT.append(np.ascontiguousarray(xs.T.astype(bf)))       # [128F, CN]
        gid = np.where(esrc[c] >= 0, esrc[c], 0)                # [P, S]
        sup = (gid // NPR).astype(np.int16)
        w16 = sup.T.reshape(-1).reshape(-1, 16)                 # [S*8, 16]
        idx = np.empty((128, S * 8), np.int16)
        for g in range(8):
            idx[g * 16:(g + 1) * 16, :] = w16.T
        gidx16.append(idx)
        wmod = gid % NPR
        msk = np.zeros((P, NPR * S), dtype=bf)
        for k in range(NPR):
            # pad slots (esrc<0) get all-zero masks -> exact 0 after select
            msk[:, k * S:(k + 1) * S] = (wmod == k) & (esrc[c] >= 0)
        mskE.append(msk)
    return dict(xET=xET, E1E=E1E, E2E=E2E, xsT=xsT, gidx16=gidx16,
                mskE=mskE, old_of_new=old_of_new), meta


def build_program(meta):
    NB, CN, S = meta["NB"], meta["CN"], meta["S"]
    NTOT = meta["NTOT"]
    NROWS = NTOT // NPR                 # 25088 table rows + 1 dummy
    SH = CN // NPR                      # shard rows per core
    Lb, offs, packs = meta["Lb"], meta["offs"], meta["packs"]
    EXP = mybir.ActivationFunctionType.Exp

    nc = bacc.Bacc("TRN2", target_bir_lowering=False, debug=False,
                   num_devices=NCORES)

    xET_d = nc.declare_dram_parameter("xET", [P, S * P], BF16, isOutput=False)
    E1_d = nc.declare_dram_parameter("E1E", [P, S], BF16, isOutput=False)
    E2_d = nc.declare_dram_parameter("E2E", [P, S], BF16, isOutput=False)
    xsT_d = nc.declare_dram_parameter("xsT", [P, CN], BF16, isOutput=False)
    gidx_d = nc.declare_dram_parameter("gidx16", [P, S * 8], I16,
                                       isOutput=False)
    msk_d = nc.declare_dram_parameter("mskE", [P, NPR * S], BF16,
                                      isOutput=False)
    w1_d = nc.declare_dram_parameter("w1", [P, HID], BF16, isOutput=False)
    wl1_d = nc.declare_dram_parameter("wl1", [P, HID + 1], BF16, isOutput=False)
    w2_d = nc.declare_dram_parameter("w2", [HID, OUT + 4], BF16, isOutput=False)
    bc1_d = nc.declare_dram_parameter("bc1", [1, HID], F32, isOutput=False)
    bc2_d = nc.declare_dram_parameter("bc2", [1, OUT], F32, isOutput=False)
    out_d = nc.declare_dram_parameter("out", [CN, OUT], F32, isOutput=True)

    tbl4s = nc.dram_tensor("tbl4s", [SH, ROWE], F32)
    tbl4g = nc.dram_tensor("tbl4g", [NROWS, ROWE], F32)

    def ap(t, off, dims):
        return bass.AP(t[:].tensor, off, dims)

    with tile.TileContext(nc) as tc:
        with (
            tc.tile_pool(name="res", bufs=1) as res,
            tc.tile_pool(name="wrk", bufs=3) as wrk,
            tc.tile_pool(name="big", bufs=2) as big,
            tc.tile_pool(name="ps", bufs=3, space="PSUM") as psp,
            tc.tile_pool(name="ps2", bufs=2, space="PSUM") as psp2,
        ):
            w1_sb = res.tile([P, HID], BF16)
            nc.sync.dma_start(w1_sb[:], w1_d[:])
            wl1_sb = res.tile([P, HID + 1], BF16)
            nc.sync.dma_start(wl1_sb[:], wl1_d[:])
            w2_sb = res.tile([HID, OUT + 4], BF16)
            nc.sync.dma_start(w2_sb[:], w2_d[:])
            bc1_sb = res.tile([P, HID], F32)
            nc.sync.dma_start(bc1_sb[:], ap(bc1_d, 0, [[0, P], [1, HID]]))
            bc2_sb = res.tile([P, OUT], F32)
            nc.sync.dma_start(bc2_sb[:], ap(bc2_d, 0, [[0, P], [1, OUT]]))
            ident = res.tile([P, P], F32)
            make_identity(nc, ident[:])
            E1sb = res.tile([P, S], BF16)
            nc.sync.dma_start(E1sb[:], E1_d[:])
            E2sb = res.tile([P, S], BF16)
            nc.sync.dma_start(E2sb[:], E2_d[:])
            gidx_sb = res.tile([P, S * 8], I16)
            nc.sync.dma_start(gidx_sb[:], gidx_d[:])
            msk_sb = res.tile([P, NPR * S], BF16)
            nc.sync.dma_start(msk_sb[:], msk_d[:])

            linbuf = res.tile([P, NB, HID], F32)
            lin2buf = res.tile([P, NB, OUT], F32)
            edl = res.tile([P, NB], F32)
            R1 = res.tile([P, NB], F32)
            R2 = res.tile([P, NB], F32)
            ed2l = res.tile([P, NB], F32)
            R21 = res.tile([P, NB], F32)
            R22 = res.tile([P, NB], F32)
            s1 = res.tile([P, NB], F32)
            s2 = res.tile([P, NB], F32)
            hT = res.tile([HID, CN], BF16)
            outsb = res.tile([P, NB, OUT], F32)
            G2 = res.tile([P, S, TW2], BF16)

            # ---- phase A: shard lin1 / ed1 -------------------------------
            for b in range(NB):
                xs_sb = wrk.tile([P, P], BF16, tag="xs")
                nc.sync.dma_start(xs_sb[:], xsT_d[:, b * P:(b + 1) * P])
                psB = psp.tile([P, SUBB * HID], F32, tag="ps")
                nc.tensor.matmul(psB[:, 0:HID + 1], xs_sb[:], wl1_sb[:],
                                 start=True, stop=True)
                nc.vector.tensor_tensor(out=linbuf[:, b, :],
                                        in0=psB[:, 0:HID], in1=bc1_sb[:],
                                        op=mybir.AluOpType.add)
                nc.scalar.copy(edl[:, b:b + 1], psB[:, HID:HID + 1])
            nc.scalar.activation(R1[:], edl[:], EXP)
            nc.scalar.activation(R2[:], edl[:], EXP, scale=0.2)

            # ---- phase B: layer 1 ----------------------------------------
            for col0, blocks in packs:
                cols = sum(Lb[b] for b in blocks)
                G = big.tile([P, PACK, HID], BF16, tag="G")
                for c0 in range(0, cols, SUBB):
                    nsub = min(SUBB, cols - c0)
                    xe_sb = wrk.tile([P, SUBB * P], BF16, tag="xe")
                    nc.sync.dma_start(
                        xe_sb[:, 0:nsub * P],
                        xET_d[:, (col0 + c0) * P:(col0 + c0 + nsub) * P])
                    psA = psp.tile([P, SUBB * HID], F32, tag="ps")
                    for j in range(nsub):
                        nc.tensor.matmul(psA[:, j * HID:(j + 1) * HID],
                                         xe_sb[:, j * P:(j + 1) * P],
                                         w1_sb[:], start=True, stop=True)
                    nc.scalar.copy(
                        bass.AP(G[:].tensor, G[:].offset + c0 * HID,
                                [G[:].ap[0], [1, nsub * HID]]),
                        psA[:, 0:nsub * HID])
                Pp = wrk.tile([P, PACK], BF16, tag="Pp")
                for b in blocks:
                    o, L = offs[b], Lb[b]
                    oo = o - col0
                    t1 = wrk.tile([P, PACK], F32, tag="t1")
                    nc.vector.scalar_tensor_tensor(
                        out=t1[:, 0:L], in0=E2sb[:, o:o + L],
                        scalar=R2[:, b:b + 1], in1=E2sb[:, o:o + L],
                        op0=mybir.AluOpType.mult,
                        op1=mybir.AluOpType.bypass)
                    nc.vector.scalar_tensor_tensor(
                        out=Pp[:, oo:oo + L], in0=E1sb[:, o:o + L],
                        scalar=R1[:, b:b + 1], in1=t1[:, 0:L],
                        op0=mybir.AluOpType.mult, op1=mybir.AluOpType.max,
                        accum_out=s1[:, b:b + 1])
                W = big.tile([P, PACK, HID], BF16, tag="W")
                nc.vector.tensor_tensor(
                    out=W[:, 0:cols, :], in0=G[:, 0:cols, :],
                    in1=bass.AP(Pp[:].tensor, Pp[:].offset,
                                [Pp[:].ap[0], [1, cols], [0, HID]]),
                    op=mybir.AluOpType.mult)
                for b in blocks:
                    o, L = offs[b], Lb[b]
                    oo = o - col0
                    # contiguous tree reduction over the L slot columns
                    n = L
                    while n > 1:
                        h = n // 2
                        nc.vector.tensor_tensor(
                            out=bass.AP(W[:].tensor, W[:].offset + oo * HID,
                                        [W[:].ap[0], [1, h * HID]]),
                            in0=bass.AP(W[:].tensor, W[:].offset + oo * HID,
                                        [W[:].ap[0], [1, h * HID]]),
                            in1=bass.AP(W[:].tensor,
                                        W[:].offset + (oo + n - h) * HID,
                                        [W[:].ap[0], [1, h * HID]]),
                            op=mybir.AluOpType.add)
                        n -= h
                    acc = wrk.tile([P, HID], F32, tag="acc")
                    nc.vector.tensor_copy(
                        acc[:], bass.AP(W[:].tensor, W[:].offset + oo * HID,
                                        [W[:].ap[0], [1, HID]]))
                    rec = wrk.tile([P, 1], F32, tag="rec")
                    nc.vector.reciprocal(rec[:], s1[:, b:b + 1])
                    th = wrk.tile([P, HID], F32, tag="th")
                    nc.vector.scalar_tensor_tensor(
                        out=th[:], in0=acc[:], scalar=rec[:, 0:1],
                        in1=linbuf[:, b, :], op0=mybir.AluOpType.mult,
                        op1=mybir.AluOpType.add)
                    psT = psp2.tile([HID, P], F32, tag="pst")
                    nc.tensor.transpose(out=psT[:], in_=th[:],
                                        identity=ident[:])
                    nc.scalar.activation(hT[:, b * P:(b + 1) * P], psT[:],
                                         mybir.ActivationFunctionType.Relu)

            # ---- phase C: layer-2 table ----------------------------------
            for b in range(NB):
                psC = psp.tile([P, SUBB * HID], F32, tag="ps")
                nc.tensor.matmul(psC[:, 0:OUT + 4],
                                 hT[:, b * P:(b + 1) * P], w2_sb[:],
                                 start=True, stop=True)
                e4 = wrk.tile([P, EW], F32, tag="e4")
                nc.vector.tensor_copy(e4[:, 0:2], psC[:, 0:2])
                nc.scalar.activation(e4[:, 2:3], psC[:, 2:3], EXP)
                nc.scalar.activation(e4[:, 3:4], psC[:, 2:3], EXP, scale=0.2)
                nc.vector.memset(e4[:, TW2:EW], 0.0)
                nc.sync.dma_start(
                    ap(tbl4s, b * P * EW, [[EW, P], [1, EW]]), e4[:])
                nc.scalar.copy(ed2l[:, b:b + 1], psC[:, OUT + 1:OUT + 2])
                nc.vector.tensor_tensor(out=lin2buf[:, b, :],
                                        in0=psC[:, OUT + 2:OUT + 4],
                                        in1=bc2_sb[:],
                                        op=mybir.AluOpType.add)
            nc.scalar.activation(R21[:], ed2l[:], EXP)
            nc.scalar.activation(R22[:], ed2l[:], EXP, scale=0.2)
            nc.gpsimd.collective_compute(
                "AllGather", mybir.AluOpType.bypass,
                replica_groups=[list(range(NCORES))],
                ins=[tbl4s[:]], outs=[tbl4g[:]])

            # ---- phase D: layer 2, attention interleaved into the gather -
            P2f = res.tile([P, S], BF16)
            done_b, done_pk = 0, 0
            for col0 in range(0, S, CC):
                kc = min(CC, S - col0)
                gbuf = big.tile([P, CC, ROWE], F32, tag="gb")
                for g0 in range(0, kc, GC):
                    gk = min(GC, kc - g0)
                    nc.gpsimd.dma_gather(
                        out_ap=gbuf[:, g0:g0 + gk, :], in_ap=tbl4g[:],
                        idxs_ap=gidx_sb[:, (col0 + g0) * 8:
                                        (col0 + g0 + gk) * 8],
                        num_idxs=gk * P, num_idxs_reg=gk * P,
                        elem_size=ROWE)
                g2o = bass.AP(G2[:].tensor, G2[:].offset + col0 * TW2,
                              [G2[:].ap[0], [TW2, kc], [1, TW2]])
                for k in range(NPR):
                    src = bass.AP(gbuf[:].tensor, gbuf[:].offset + k * EW,
                                  [gbuf[:].ap[0], [ROWE, kc], [1, TW2]])
                    mk = bass.AP(msk_sb[:].tensor,
                                 msk_sb[:].offset + k * S + col0,
                                 [msk_sb[:].ap[0], [1, kc], [0, TW2]])
                    if k == 0:
                        nc.vector.tensor_tensor(out=g2o, in0=src, in1=mk,
                                                op=mybir.AluOpType.mult)
                    else:
                        tt = wrk.tile([P, CC * TW2], F32, tag="tt")
                        tv = bass.AP(tt[:].tensor, tt[:].offset,
                                     [tt[:].ap[0], [TW2, kc], [1, TW2]])
                        nc.vector.tensor_tensor(out=tv, in0=src, in1=mk,
                                                op=mybir.AluOpType.mult)
                        nc.vector.tensor_tensor(out=g2o, in0=g2o, in1=tv,
                                                op=mybir.AluOpType.add)
                chunk_end = col0 + kc
                while done_b < NB and offs[done_b] + Lb[done_b] <= chunk_end:
                    b = done_b
                    o, L = offs[b], Lb[b]
                    q1v = bass.AP(G2[:].tensor, G2[:].offset + o * TW2 + 2,
                                  [G2[:].ap[0], [TW2, L]])
                    q2v = bass.AP(G2[:].tensor, G2[:].offset + o * TW2 + 3,
                                  [G2[:].ap[0], [TW2, L]])
                    t1 = wrk.tile([P, PACK], F32, tag="t1")
                    nc.vector.scalar_tensor_tensor(
                        out=t1[:, 0:L], in0=q2v, scalar=R22[:, b:b + 1],
                        in1=q2v, op0=mybir.AluOpType.mult,
                        op1=mybir.AluOpType.bypass)
                    nc.vector.scalar_tensor_tensor(
                        out=P2f[:, o:o + L], in0=q1v,
                        scalar=R21[:, b:b + 1], in1=t1[:, 0:L],
                        op0=mybir.AluOpType.mult, op1=mybir.AluOpType.max,
                        accum_out=s2[:, b:b + 1])
                    nc.vector.tensor_scalar_max(s2[:, b:b + 1],
                                                s2[:, b:b + 1], 1e-30)
                    done_b += 1
                while done_pk < len(packs):
                    col0p, blocksp = packs[done_pk]
                    colsp = sum(Lb[b] for b in blocksp)
                    if col0p + colsp > chunk_end or \
                            blocksp[-1] >= done_b:
                        break
                    W2t = wrk.tile([P, PACK, OUT], F32, tag="W2t")
                    nc.vector.tensor_tensor(
                        out=W2t[:, 0:colsp, :],
                        in0=bass.AP(G2[:].tensor,
                                    G2[:].offset + col0p * TW2,
                                    [G2[:].ap[0], [TW2, colsp], [1, OUT]]),
                        in1=bass.AP(P2f[:].tensor, P2f[:].offset + col0p,
                                    [P2f[:].ap[0], [1, colsp], [0, OUT]]),
                        op=mybir.AluOpType.mult)
                    for b in blocksp:
                        o, L = offs[b], Lb[b]
                        oo = o - col0p
                        acc2 = wrk.tile([P, OUT], F32, tag="acc2")
                        wv = bass.AP(W2t[:].tensor, W2t[:].offset + oo * OUT,
                                     [W2t[:].ap[0], [1, OUT], [OUT, L]])
                        nc.vector.tensor_reduce(out=acc2[:], in_=wv,
                                                axis=mybir.AxisListType.X,
                                                op=mybir.AluOpType.add)
                        rec = wrk.tile([P, 1], F32, tag="rec")
                        nc.vector.reciprocal(rec[:], s2[:, b:b + 1])
                        to = wrk.tile([P, OUT], F32, tag="to")
                        nc.vector.scalar_tensor_tensor(
                            out=to[:], in0=acc2[:], scalar=rec[:, 0:1],
                            in1=lin2buf[:, b, :], op0=mybir.AluOpType.mult,
                            op1=mybir.AluOpType.add)
                        nc.scalar.activation(
                            outsb[:, b, :], to[:],
                            mybir.ActivationFunctionType.Sigmoid)
                    done_pk += 1

            nc.sync.dma_start(
                ap(out_d, 0, [[OUT, P], [OUT * P, NB], [1, OUT]]), outsb[:])

    nc.compile()
    return nc


def _host_params(W1_src, att1_src, W1_dst, att1_dst, b1, Wl1, bl1,
                 W2_src, att2_src, W2_dst, att2_dst, b2, Wl2, bl2):
    bf = ml_dtypes.bfloat16
    v1s = (np.asarray(W1_src, np.float64)
           @ np.asarray(att1_src, np.float64)[0]).astype(np.float32)
    v1d = (W1_dst @ att1_dst[0]).astype(np.float32)
    v2s = (W2_src @ att2_src[0]).astype(np.float32)
    v2d = (W2_dst @ att2_dst[0]).astype(np.float32)
    # w2 cols: hs2_0 hs2_1 es2 ed2 lin2_0 lin2_1  -> OUT+4 wide
    w2 = np.concatenate([W2_src, v2s[:, None], v2d[:, None], Wl2], axis=1)
    wl1 = np.concatenate([Wl1, v1d[:, None]], axis=1)
    return dict(
        w1=np.asarray(W1_src).astype(bf), wl1=wl1.astype(bf),
        w2=w2.astype(bf),
        bc1=(b1 + bl1).reshape(1, HID).astype(np.float32),
        bc2=(b2 + bl2).reshape(1, OUT).astype(np.float32)), v1s


_CACHE = {}


def run(x, edge_index, params, cfg, runner=None):
    pp, v1s = _host_params(**params)
    host, meta = preprocess(x, edge_index, v1s, cfg)
    key = (tuple(meta["Lb"]), meta["CN"])
    if key not in _CACHE:
        _CACHE[key] = build_program(meta)
    nc = _CACHE[key]
    in_maps = []
    for c in range(NCORES):
        m = dict(pp)
        m["xET"] = host["xET"][c]
        m["E1E"] = host["E1E"][c]
        m["E2E"] = host["E2E"][c]
        m["xsT"] = host["xsT"][c]
        m["gidx16"] = host["gidx16"][c]
        m["mskE"] = host["mskE"][c]
        in_maps.append(m)
    if runner is None:
        res = run_bass_kernel_spmd(nc, in_maps, list(range(NCORES)))
        outs = [r["out"] for r in res.results]
    else:
        outs, res = runner(nc, in_maps)
    full = np.concatenate(outs, axis=0)
    y = np.zeros((cfg["N"], OUT), dtype=np.float32)
    valid = host["old_of_new"] >= 0
    y[host["old_of_new"][valid]] = full[valid]
    return y, res


def kernel(x, edge_index, W1_src, W1_dst, att1_src, att1_dst, b1, Wl1, bl1,
           W2_src, W2_dst, att2_src, att2_dst, b2, Wl2, bl2):
    cfg = dict(N=100000, CN=12544, NB=98)
    params = dict(W1_src=np.asarray(W1_src), att1_src=np.asarray(att1_src),
                  W1_dst=np.asarray(W1_dst), att1_dst=np.asarray(att1_dst),
                  b1=np.asarray(b1), Wl1=np.asarray(Wl1), bl1=np.asarray(bl1),
                  W2_src=np.asarray(W2_src), att2_src=np.asarray(att2_src),
                  W2_dst=np.asarray(W2_dst), att2_dst=np.asarray(att2_dst),
                  b2=np.asarray(b2), Wl2=np.asarray(Wl2), bl2=np.asarray(bl2))
    y, _ = run(np.asarray(x), np.asarray(edge_index), params, cfg)
    return y
